# revision 16
# baseline (speedup 1.0000x reference)
"""AnchorDML Trainium2 kernel: 8-core SPMD, data-parallel over x rows with
sharded anchor encoding + AllGather of encoded anchors.

Problem (hardcoded):
    N, M, D, C = 8192, 4096, 512, 100
    xe = mish(mish(x @ W1 + b1) @ W2 + b2)          [N, D]
    se = mish(mish(samples @ W1 + b1) @ W2 + b2)    [M, D]
    dist = sqrt(max(|xe|^2 + |se|^2 - 2 xe@se.T, 0))  [N, M]
    out = log_softmax(tanh(dist @ Wp + bp), axis=1)   [N, C]

Sharding: core g handles x rows [1024g, 1024(g+1)) and encodes anchors
[512g, 512(g+1)); encoded (scaled) anchors + |se|^2 are AllGathered.

The per-core encoder input is the column-concatenation [sT | xT] so both
encodes share one instruction stream (fewer ACT table switches); layer 2
runs the anchor columns first so the AllGather is issued as early as
possible and overlaps the x-side encode.

Precision: encoder and distance GEMM operands are bf16 (fp32 psum
accumulation; errors enter via operand rounding only and add in
quadrature through the distance, ~2e-3 of output scale). The perceptron
GEMM (dist @ Wp) stays float32r because dist ~ 32 is nearly constant, so
Wp rounding would bias whole output columns. |xe|^2 / |se|^2 ride in
fp32 via one DVE pass per distance tile.

mish(v) = v * tanh(ln(1 + e^v)) from exp/ln/tanh LUTs (no mish LUT in
this compiler build); tanh is batched per layer phase to amortize ACT
table loads; the pre-activation v is staged out of PSUM immediately so
banks recycle fast.
"""
import numpy as np
import ml_dtypes
from concourse import bass, bacc, tile, mybir, bass_utils, masks

N, M, D, C = 8192, 4096, 512, 100
NCORES = 8
RPC = N // NCORES      # 1024 x-rows per core
MPC = M // NCORES      # 512 anchors encoded per core
EW = MPC + RPC         # 1536 merged encoder columns
KD = D // 128          # 4 contraction chunks of 128
NMT = M // 128         # 32 anchor tiles in the distance matmul
NRC = RPC // 512       # 2 row-chunks of 512

F32 = mybir.dt.float32
F32R = mybir.dt.float32r
BF16 = mybir.dt.bfloat16
AF = mybir.ActivationFunctionType
ALU = mybir.AluOpType


def _patched_tables(arch):
    """Subset the ACT table sets (keeping dict order — act_func_set_id is
    positional) so Exp/Ln resolve only to natural_log_exp_and_others and
    Tanh only to exp_and_others. The default first-match choice alternates
    exp_and_others <-> natural_log on every exp/ln pair, paying a 1.3us
    table load each time."""
    from concourse.hw_specs import get_activation_tables as orig
    out = {}
    for name, s in orig(arch).items():
        s = set(s)
        if name != "natural_log_exp_and_others":
            s.discard(AF.Exp)
            s.discard(AF.Ln)
        if name != "exp_and_others":
            s.discard(AF.Tanh)
        out[name] = s
    return out


def build_kernel():
    bacc.get_activation_tables = _patched_tables
    nc = bacc.Bacc("TRN2", target_bir_lowering=False, debug=False,
                   num_devices=NCORES)

    eT = nc.dram_tensor("eT", [D, EW], BF16, kind="ExternalInput")
    W1 = nc.dram_tensor("W1", [D, D], BF16, kind="ExternalInput")
    W2 = nc.dram_tensor("W2", [D, D], BF16, kind="ExternalInput")
    b1 = nc.dram_tensor("b1", [D, 1], F32, kind="ExternalInput")
    b2 = nc.dram_tensor("b2", [D, 1], F32, kind="ExternalInput")
    Wp = nc.dram_tensor("Wp", [M, C], F32, kind="ExternalInput")
    bp = nc.dram_tensor("bp", [1, C], F32, kind="ExternalInput")
    out = nc.dram_tensor("out", [RPC, C], F32, kind="ExternalOutput")

    with tile.TileContext(nc) as tc:
        _body(tc, eT, W1, W2, b1, b2, Wp, bp, out)

    nc.compile()
    return nc


def _body(tc, eT, W1, W2, b1, b2, Wp, bp, out):
    nc = tc.nc
    with (
        tc.tile_pool(name="const", bufs=1) as const,
        tc.tile_pool(name="wpool", bufs=1) as wpool,
        tc.tile_pool(name="spool", bufs=1) as spool,
        tc.tile_pool(name="xpool", bufs=1) as xpool,
        tc.tile_pool(name="gpool", bufs=1) as gpool,
        tc.tile_pool(name="mpool", bufs=2) as mpool,
        tc.tile_pool(name="dpool", bufs=6) as dpool,
        tc.tile_pool(name="zpool", bufs=2) as zpool,
        tc.tile_pool(name="ps", bufs=1, space="PSUM") as ps,
        tc.tile_pool(name="psz", bufs=1, space="PSUM") as psz,
        tc.tile_pool(name="dram", bufs=1, space="DRAM") as dram,
    ):
        # ---- first-needed input loads ----
        eT_sb = xpool.tile([128, KD, EW], BF16)
        for k in range(KD):
            nc.sync.dma_start(eT_sb[:, k, :], eT[128 * k:128 * (k + 1), :])
        W1_sb = wpool.tile([128, KD, D], BF16)
        for k in range(KD):
            nc.sync.dma_start(W1_sb[:, k, :], W1[128 * k:128 * (k + 1), :])
        b1c_sb = wpool.tile([128, KD], F32)
        b2c_sb = wpool.tile([128, KD], F32)
        for k in range(KD):
            nc.sync.dma_start(b1c_sb[:, k:k + 1], b1[128 * k:128 * (k + 1), :])
            nc.sync.dma_start(b2c_sb[:, k:k + 1], b2[128 * k:128 * (k + 1), :])
        W2_sb = wpool.tile([128, KD, D], BF16)
        for k in range(KD):
            nc.sync.dma_start(W2_sb[:, k, :], W2[128 * k:128 * (k + 1), :])

        # ---- constants ----
        ident = const.tile([C, C], F32)
        masks.make_identity(nc, ident[:])
        ones_f32 = const.tile([128, 1], F32)
        nc.gpsimd.memset(ones_f32[:], 1.0)
        ones_col = const.tile([128, 1], BF16)    # lhsT for row-sum matmuls
        nc.scalar.activation(ones_col[:], ones_f32[:], AF.Copy)
        onesr_f32 = const.tile([1, 512], F32)
        nc.gpsimd.memset(onesr_f32[:], 1.0)
        ones512 = const.tile([1, 512], F32R)     # rhs/lhsT for rank-1 terms
        nc.scalar.activation(ones512[:], onesr_f32[:], AF.Copy)

        # later-needed weights
        bp_sb = wpool.tile([1, C], F32R)
        nc.sync.dma_start(bp_sb[:], bp[:].bitcast(F32R))
        Wp_sb = wpool.tile([128, NMT, C], F32R)
        for t in range(NMT):
            nc.sync.dma_start(Wp_sb[:, t, :],
                              Wp[128 * t:128 * (t + 1), :].bitcast(F32R))

        def enc_phase(dst, dst_off, Wsb, bcol, src, src_off, width):
            """dst[:, :, dst_off:dst_off+width] = mish(src.T @ W + b) for all
            KD feature chunks x (width/512) column chunks of one layer phase.
            v is staged (with bias) so psum recycles fast; sp=ln(1+e^v) lands
            in dst; tanh + v*t multiply are batched over the whole phase."""
            nw = width // 512
            vstage = mpool.tile([128, KD, 1536], BF16, tag="vstage")
            for w in range(nw):
                ssl = slice(src_off + 512 * w, src_off + 512 * (w + 1))
                for f in range(KD):
                    vps = ps.tile([128, 512], F32, tag="mm", bufs=4)
                    for k in range(KD):
                        nc.tensor.matmul(vps[:],
                                         Wsb[:, k, 128 * f:128 * (f + 1)],
                                         src[:, k, ssl],
                                         start=(k == 0), stop=(k == KD - 1))
                    u = mpool.tile([128, 512], BF16, tag="mu", bufs=3)
                    nc.scalar.activation(u[:], vps[:], AF.Exp,
                                         bias=bcol[:, f:f + 1])
                    nc.vector.tensor_scalar_add(
                        vstage[:, f, 512 * w:512 * (w + 1)], vps[:],
                        bcol[:, f:f + 1])
                    nc.scalar.activation(
                        dst[:, f, dst_off + 512 * w:dst_off + 512 * (w + 1)],
                        u[:], AF.Ln, bias=1.0)
            dsl = slice(dst_off, dst_off + width)
            nc.scalar.activation(dst[:, :, dsl], dst[:, :, dsl], AF.Tanh)
            nc.vector.tensor_tensor(dst[:, :, dsl],
                                    vstage[:, :, :width],
                                    dst[:, :, dsl], op=ALU.mult)

        # ---- layer 1 over merged [anchors | x] columns ----
        h_all = xpool.tile([128, KD, EW], BF16)
        enc_phase(h_all, 0, W1_sb, b1c_sb, eT_sb, 0, EW)

        # ---- layer 2, anchor columns first (releases the AllGather) ----
        se_sb = spool.tile([128, KD, MPC], BF16)
        enc_phase(se_sb, 0, W2_sb, b2c_sb, h_all, 0, MPC)

        # s2 row, seA = -2*se, and the (tiny-first) AllGathers
        sqse_sb = spool.tile([128, KD, MPC], BF16)
        nc.vector.tensor_tensor(sqse_sb[:, :, :], se_sb[:, :, :],
                                se_sb[:, :, :], op=ALU.mult)
        s2ps = ps.tile([1, 512], F32, tag="tr", bufs=2)
        for k in range(KD):
            nc.tensor.matmul(s2ps[:], ones_col[:], sqse_sb[:, k, :],
                             start=(k == 0), stop=(k == KD - 1))
        s2row_sb = spool.tile([1, MPC], F32)
        nc.vector.tensor_copy(s2row_sb[:], s2ps[:])
        seA_sb = spool.tile([128, KD, MPC], BF16, tag="sqse_sb")
        nc.vector.tensor_scalar_mul(seA_sb[:, :, :], se_sb[:, :, :], -2.0)

        # one collective: [seA (bf16, 512 rows) ; s2 (f32 packed as 2 rows)]
        AGR = D + 2
        ag_in = dram.tile([AGR, MPC], BF16)
        ag_out = dram.tile([NCORES * AGR, MPC], BF16, addr_space="Shared")
        for k in range(KD):
            nc.scalar.dma_start(ag_in[128 * k:128 * (k + 1), :],
                                seA_sb[:, k, :])
        nc.scalar.dma_start(
            ag_in[D:D + 2, :].rearrange("(o a) b -> o (a b)", o=1),
            s2row_sb[:].bitcast(BF16))
        nc.gpsimd.collective_compute(
            "AllGather", ALU.bypass,
            replica_groups=[list(range(NCORES))],
            ins=[ag_in.opt()], outs=[ag_out.opt()])

        # ---- layer 2, x columns (overlaps the AllGather) ----
        xe_sb = xpool.tile([128, KD, RPC], BF16, tag="eT_sb")
        enc_phase(xe_sb, 0, W2_sb, b2c_sb, h_all, MPC, RPC)

        # x2 broadcast tile: x2b[p, rc, r] = |xe_r|^2 for every partition
        sqxe_sb = xpool.tile([128, KD, RPC], BF16, tag="h_all")
        nc.vector.tensor_tensor(sqxe_sb[:, :, :], xe_sb[:, :, :],
                                xe_sb[:, :, :], op=ALU.mult)
        x2row_sb = xpool.tile([1, RPC], F32R)
        x2b_sb = xpool.tile([128, NRC, 512], F32)
        for rc in range(NRC):
            xps = ps.tile([1, 512], F32, tag="tr", bufs=2)
            for k in range(KD):
                nc.tensor.matmul(xps[:], ones_col[:],
                                 sqxe_sb[:, k, 512 * rc:512 * (rc + 1)],
                                 start=(k == 0), stop=(k == KD - 1))
            nc.vector.tensor_copy(x2row_sb[:, 512 * rc:512 * (rc + 1)],
                                  xps[:])
            xbs = ps.tile([128, 512], F32, tag="mm", bufs=4)
            nc.tensor.matmul(xbs[:], ones512[:, :128],
                             x2row_sb[:, 512 * rc:512 * (rc + 1)],
                             start=True, stop=True)
            nc.vector.tensor_copy(x2b_sb[:, rc, :], xbs[:])

        # ---- load gathered anchors: seAg[d_part, k, m] for all 4096 m ----
        # s2 columns first (they gate the first distance tile):
        # s2c[p, t] = s2[128 t + p]
        s2c_sb = gpool.tile([128, NMT], F32)
        for g in range(NCORES):
            rows = ag_out[AGR * g + D:AGR * g + D + 2, :].bitcast(F32)
            nc.scalar.dma_start(
                s2c_sb[:, (MPC // 128) * g:(MPC // 128) * (g + 1)],
                rows.rearrange("a (f p) -> p (a f)", f=2, p=128))
        seAg_sb = gpool.tile([128, KD, M], BF16)
        for g in range(NCORES):
            for k in range(KD):
                nc.sync.dma_start(
                    seAg_sb[:, k, MPC * g:MPC * (g + 1)],
                    ag_out[AGR * g + 128 * k:AGR * g + 128 * (k + 1), :])

        # ---- main fused loop: distances + perceptron (transposed),
        # row-chunk-outer so rc0's epilogue overlaps rc1's GEMMs. The zT
        # matmul for tile t is emitted after the d2 group of tile t+2 so
        # the in-order PE stream never waits on the sqrt pass. ----
        for rc in range(NRC):
            zt_ps = psz.tile([C, 512], F32, name=f"ztps{rc}")
            dist_tiles = {}
            for t in range(NMT):
                d2ps = ps.tile([128, 512], F32, tag="mm", bufs=4)
                for k in range(KD):
                    nc.tensor.matmul(d2ps[:],
                                     seAg_sb[:, k, 128 * t:128 * (t + 1)],
                                     xe_sb[:, k, 512 * rc:512 * (rc + 1)],
                                     start=(k == 0), stop=(k == KD - 1))
                # d2 += s2[m] (per-partition) + x2[r] (broadcast row), fp32
                nc.vector.scalar_tensor_tensor(
                    d2ps[:], d2ps[:], s2c_sb[:, t:t + 1], x2b_sb[:, rc, :],
                    op0=ALU.add, op1=ALU.add)
                distT = dpool.tile([128, 512], F32R)
                nc.scalar.activation(distT[:], d2ps[:], AF.Sqrt)
                dist_tiles[t] = distT
                if t >= 2:
                    nc.tensor.matmul(zt_ps[:], Wp_sb[:, t - 2, :],
                                     dist_tiles.pop(t - 2)[:],
                                     start=(t - 2 == 0), stop=False,
                                     skip_group_check=True)
            for t in (NMT - 2, NMT - 1):
                nc.tensor.matmul(zt_ps[:], Wp_sb[:, t, :],
                                 dist_tiles.pop(t)[:],
                                 start=False, stop=False,
                                 skip_group_check=True)
            # epilogue for this row chunk: bias, transpose, tanh+log-softmax
            nc.tensor.matmul(zt_ps[:], bp_sb[:], ones512[:],
                             start=False, stop=True, skip_group_check=True)
            zt_sb = zpool.tile([C, 512], F32, bufs=2, tag="zt")
            nc.vector.tensor_copy(zt_sb[:], zt_ps[:])
            zpre = zpool.tile([128, 4, C], BF16, bufs=2, tag="zpre")
            for j in range(4):
                ztr = ps.tile([128, C], F32, tag="tr", bufs=2)
                nc.tensor.matmul(ztr[:], zt_sb[:, 128 * j:128 * (j + 1)],
                                 ident[:], is_transpose=True)
                nc.vector.tensor_copy(zpre[:, j, :], ztr[:])
            zth = zpool.tile([128, 4, C], BF16, bufs=2, tag="zth")
            nc.scalar.activation(zth[:, :, :], zpre[:, :, :], AF.Tanh)
            e_sb = zpool.tile([128, 4, C], BF16, bufs=2, tag="esb")
            nc.scalar.activation(e_sb[:, :, :], zth[:, :, :], AF.Exp)
            ssum = zpool.tile([128, 4], F32, bufs=2, tag="ssum")
            nc.vector.tensor_reduce(ssum[:], e_sb[:, :, :],
                                    axis=mybir.AxisListType.X, op=ALU.add)
            lns = zpool.tile([128, 4], F32, bufs=2, tag="lns")
            nc.scalar.activation(lns[:], ssum[:], AF.Ln)
            for j in range(4):
                o_sb = zpool.tile([128, C], F32, bufs=2, tag="osb")
                nc.vector.tensor_scalar(o_sb[:], zth[:, j, :],
                                        lns[:, j:j + 1], None,
                                        op0=ALU.subtract)
                nc.sync.dma_start(
                    out[512 * rc + 128 * j:512 * rc + 128 * (j + 1), :],
                    o_sb[:])


_NC_CACHE = None


def _get_nc():
    global _NC_CACHE
    if _NC_CACHE is None:
        _NC_CACHE = build_kernel()
    return _NC_CACHE


def make_in_maps(x, samples, W1, b1, W2, b2, Wp, bp):
    bf = ml_dtypes.bfloat16
    x = np.asarray(x, dtype=np.float32)
    samples = np.asarray(samples, dtype=np.float32)
    W1b = np.ascontiguousarray(np.asarray(W1, dtype=np.float32).astype(bf))
    W2b = np.ascontiguousarray(np.asarray(W2, dtype=np.float32).astype(bf))
    Wpc = np.ascontiguousarray(np.asarray(Wp, dtype=np.float32))
    b1c = np.ascontiguousarray(np.asarray(b1, dtype=np.float32).reshape(D, 1))
    b2c = np.ascontiguousarray(np.asarray(b2, dtype=np.float32).reshape(D, 1))
    bpc = np.ascontiguousarray(np.asarray(bp, dtype=np.float32).reshape(1, C))
    in_maps = []
    for g in range(NCORES):
        sT_g = samples[MPC * g:MPC * (g + 1), :].T
        xT_g = x[RPC * g:RPC * (g + 1), :].T
        eT_g = np.concatenate([sT_g, xT_g], axis=1).astype(bf)
        in_maps.append({
            "eT": np.ascontiguousarray(eT_g),
            "W1": W1b, "W2": W2b, "b1": b1c, "b2": b2c,
            "Wp": Wpc, "bp": bpc,
        })
    return in_maps


def run(in_maps, trace=False):
    nc = _get_nc()
    res = bass_utils.run_bass_kernel_spmd(nc, in_maps,
                                          core_ids=list(range(NCORES)),
                                          trace=trace)
    outp = np.concatenate([res.results[g]["out"] for g in range(NCORES)],
                          axis=0).astype(np.float32)
    return outp, res


def kernel(x, samples, W1, b1, W2, b2, Wp, bp):
    in_maps = make_in_maps(x, samples, W1, b1, W2, b2, Wp, bp)
    outp, _ = run(in_maps, trace=False)
    return outp


# revision 18
# speedup vs baseline: 1.0508x; 1.0508x over previous
"""AnchorDML Trainium2 kernel: 8-core SPMD, data-parallel over x rows with
sharded anchor encoding + AllGather of encoded anchors.

Problem (hardcoded):
    N, M, D, C = 8192, 4096, 512, 100
    xe = mish(mish(x @ W1 + b1) @ W2 + b2)          [N, D]
    se = mish(mish(samples @ W1 + b1) @ W2 + b2)    [M, D]
    dist = sqrt(max(|xe|^2 + |se|^2 - 2 xe@se.T, 0))  [N, M]
    out = log_softmax(tanh(dist @ Wp + bp), axis=1)   [N, C]

Sharding: core g handles x rows [1024g, 1024(g+1)) and encodes anchors
[512g, 512(g+1)); encoded (scaled) anchors + |se|^2 are AllGathered.

The per-core encoder input is the column-concatenation [sT | xT] so both
encodes share one instruction stream (fewer ACT table switches); layer 2
runs the anchor columns first so the AllGather is issued as early as
possible and overlaps the x-side encode.

Precision: encoder and distance GEMM operands are bf16 (fp32 psum
accumulation; errors enter via operand rounding only and add in
quadrature through the distance, ~2e-3 of output scale). The perceptron
GEMM (dist @ Wp) stays float32r because dist ~ 32 is nearly constant, so
Wp rounding would bias whole output columns. |xe|^2 / |se|^2 ride in
fp32 via one DVE pass per distance tile.

mish(v) = v * tanh(ln(1 + e^v)) from exp/ln/tanh LUTs (no mish LUT in
this compiler build); tanh is batched per layer phase to amortize ACT
table loads; the pre-activation v is staged out of PSUM immediately so
banks recycle fast.
"""
import numpy as np
import ml_dtypes
from concourse import bass, bacc, tile, mybir, bass_utils, masks

N, M, D, C = 8192, 4096, 512, 100
NCORES = 8
RPC = N // NCORES      # 1024 x-rows per core
MPC = M // NCORES      # 512 anchors encoded per core
EW = MPC + RPC         # 1536 merged encoder columns
KD = D // 128          # 4 contraction chunks of 128
NMT = M // 128         # 32 anchor tiles in the distance matmul
NRC = RPC // 512       # 2 row-chunks of 512

F32 = mybir.dt.float32
F32R = mybir.dt.float32r
BF16 = mybir.dt.bfloat16
AF = mybir.ActivationFunctionType
ALU = mybir.AluOpType


def _patched_tables(arch):
    """Subset the ACT table sets (keeping dict order — act_func_set_id is
    positional) so Exp/Ln resolve only to natural_log_exp_and_others and
    Tanh only to exp_and_others. The default first-match choice alternates
    exp_and_others <-> natural_log on every exp/ln pair, paying a 1.3us
    table load each time."""
    from concourse.hw_specs import get_activation_tables as orig
    out = {}
    for name, s in orig(arch).items():
        s = set(s)
        if name != "natural_log_exp_and_others":
            s.discard(AF.Exp)
            s.discard(AF.Ln)
        if name != "exp_and_others":
            s.discard(AF.Tanh)
        out[name] = s
    return out


def build_kernel():
    bacc.get_activation_tables = _patched_tables
    nc = bacc.Bacc("TRN2", target_bir_lowering=False, debug=False,
                   num_devices=NCORES)

    eT = nc.dram_tensor("eT", [D, EW], BF16, kind="ExternalInput")
    W1 = nc.dram_tensor("W1", [D, D], BF16, kind="ExternalInput")
    W2 = nc.dram_tensor("W2", [D, D], BF16, kind="ExternalInput")
    b1 = nc.dram_tensor("b1", [D, 1], F32, kind="ExternalInput")
    b2 = nc.dram_tensor("b2", [D, 1], F32, kind="ExternalInput")
    Wp = nc.dram_tensor("Wp", [M, C], F32, kind="ExternalInput")
    bp = nc.dram_tensor("bp", [1, C], F32, kind="ExternalInput")
    out = nc.dram_tensor("out", [RPC, C], F32, kind="ExternalOutput")

    with tile.TileContext(nc) as tc:
        _body(tc, eT, W1, W2, b1, b2, Wp, bp, out)

    nc.compile()
    return nc


def _body(tc, eT, W1, W2, b1, b2, Wp, bp, out):
    nc = tc.nc
    with (
        tc.tile_pool(name="const", bufs=1) as const,
        tc.tile_pool(name="wpool", bufs=1) as wpool,
        tc.tile_pool(name="spool", bufs=1) as spool,
        tc.tile_pool(name="xpool", bufs=1) as xpool,
        tc.tile_pool(name="gpool", bufs=1) as gpool,
        tc.tile_pool(name="mpool", bufs=2) as mpool,
        tc.tile_pool(name="dpool", bufs=6) as dpool,
        tc.tile_pool(name="zpool", bufs=2) as zpool,
        tc.tile_pool(name="ps", bufs=1, space="PSUM") as ps,
        tc.tile_pool(name="psz", bufs=1, space="PSUM") as psz,
        tc.tile_pool(name="dram", bufs=1, space="DRAM") as dram,
    ):
        # ---- first-needed input loads ----
        eT_ks, W1_ks = [], []
        for k in range(KD):
            eTk = xpool.tile([128, EW], BF16, name=f"eTk{k}")
            nc.sync.dma_start(eTk[:], eT[128 * k:128 * (k + 1), :])
            eT_ks.append(eTk)
            W1k = wpool.tile([128, D], BF16, name=f"W1k{k}")
            nc.sync.dma_start(W1k[:], W1[128 * k:128 * (k + 1), :])
            W1_ks.append(W1k)
        b1c_sb = wpool.tile([128, KD], F32)
        b2c_sb = wpool.tile([128, KD], F32)
        for k in range(KD):
            nc.sync.dma_start(b1c_sb[:, k:k + 1], b1[128 * k:128 * (k + 1), :])
            nc.sync.dma_start(b2c_sb[:, k:k + 1], b2[128 * k:128 * (k + 1), :])
        W2_ks = []
        for k in range(KD):
            W2k = wpool.tile([128, D], BF16, name=f"W2k{k}")
            nc.sync.dma_start(W2k[:], W2[128 * k:128 * (k + 1), :])
            W2_ks.append(W2k)

        # ---- constants ----
        ident = const.tile([C, C], F32)
        masks.make_identity(nc, ident[:])
        ones_f32 = const.tile([128, 1], F32)
        nc.gpsimd.memset(ones_f32[:], 1.0)
        ones_col = const.tile([128, 1], BF16)    # lhsT for row-sum matmuls
        nc.scalar.activation(ones_col[:], ones_f32[:], AF.Copy)
        onesr_f32 = const.tile([1, 512], F32)
        nc.gpsimd.memset(onesr_f32[:], 1.0)
        ones512 = const.tile([1, 512], F32R)     # rhs/lhsT for rank-1 terms
        nc.scalar.activation(ones512[:], onesr_f32[:], AF.Copy)

        # later-needed weights
        bp_sb = wpool.tile([1, C], F32R)
        nc.sync.dma_start(bp_sb[:], bp[:].bitcast(F32R))
        Wp_sb = wpool.tile([128, NMT, C], F32R)
        for t in range(NMT):
            nc.sync.dma_start(Wp_sb[:, t, :],
                              Wp[128 * t:128 * (t + 1), :].bitcast(F32R))

        def enc_phase(dst, dst_off, Wks, bcol, src_ks, src_off, width):
            """dst[:, :, dst_off:dst_off+width] = mish(src.T @ W + b) for all
            KD feature chunks x (width/512) column chunks of one layer phase.
            v is staged (with bias) so psum recycles fast; sp=ln(1+e^v) lands
            in dst; tanh + v*t multiply are batched over the whole phase."""
            nw = width // 512
            vstage = mpool.tile([128, KD, 1536], BF16, tag="vstage")
            for w in range(nw):
                ssl = slice(src_off + 512 * w, src_off + 512 * (w + 1))
                for f in range(KD):
                    vps = ps.tile([128, 512], F32, tag="mm", bufs=4)
                    for k in range(KD):
                        nc.tensor.matmul(vps[:],
                                         Wks[k][:, 128 * f:128 * (f + 1)],
                                         src_ks[k][:, ssl],
                                         start=(k == 0), stop=(k == KD - 1))
                    u = mpool.tile([128, 512], BF16, tag="mu", bufs=3)
                    nc.scalar.activation(u[:], vps[:], AF.Exp,
                                         bias=bcol[:, f:f + 1])
                    nc.vector.tensor_scalar_add(
                        vstage[:, f, 512 * w:512 * (w + 1)], vps[:],
                        bcol[:, f:f + 1])
                    nc.scalar.activation(
                        dst[:, f, dst_off + 512 * w:dst_off + 512 * (w + 1)],
                        u[:], AF.Ln, bias=1.0)
            dsl = slice(dst_off, dst_off + width)
            nc.scalar.activation(dst[:, :, dsl], dst[:, :, dsl], AF.Tanh)
            nc.vector.tensor_tensor(dst[:, :, dsl],
                                    vstage[:, :, :width],
                                    dst[:, :, dsl], op=ALU.mult)

        # ---- layer 1 over merged [anchors | x] columns ----
        h_all = xpool.tile([128, KD, EW], BF16)
        h_ks = [h_all[:, k, :] for k in range(KD)]
        enc_phase(h_all, 0, W1_ks, b1c_sb, eT_ks, 0, EW)

        # ---- layer 2, anchor columns first (releases the AllGather) ----
        se_sb = spool.tile([128, KD, MPC], BF16)
        enc_phase(se_sb, 0, W2_ks, b2c_sb, h_ks, 0, MPC)

        # s2 row, seA = -2*se, and the (tiny-first) AllGathers
        sqse_sb = spool.tile([128, KD, MPC], BF16)
        nc.gpsimd.tensor_tensor(sqse_sb[:, :, :], se_sb[:, :, :],
                                se_sb[:, :, :], op=ALU.mult)
        s2ps = ps.tile([1, 512], F32, tag="tr", bufs=2)
        for k in range(KD):
            nc.tensor.matmul(s2ps[:], ones_col[:], sqse_sb[:, k, :],
                             start=(k == 0), stop=(k == KD - 1))
        s2row_sb = spool.tile([1, MPC], F32)
        nc.scalar.activation(s2row_sb[:], s2ps[:], AF.Copy)
        seA_sb = spool.tile([128, KD, MPC], BF16, tag="sqse_sb")
        nc.gpsimd.tensor_scalar_mul(seA_sb[:, :, :], se_sb[:, :, :], -2.0)

        # one collective: [seA (bf16, 512 rows) ; s2 (f32 packed as 2 rows)]
        AGR = D + 2
        ag_in = dram.tile([AGR, MPC], BF16)
        ag_out = dram.tile([NCORES * AGR, MPC], BF16, addr_space="Shared")
        for k in range(KD):
            nc.scalar.dma_start(ag_in[128 * k:128 * (k + 1), :],
                                seA_sb[:, k, :])
        nc.scalar.dma_start(
            ag_in[D:D + 2, :].rearrange("(o a) b -> o (a b)", o=1),
            s2row_sb[:].bitcast(BF16))
        nc.gpsimd.collective_compute(
            "AllGather", ALU.bypass,
            replica_groups=[list(range(NCORES))],
            ins=[ag_in.opt()], outs=[ag_out.opt()])

        # ---- layer 2, x columns (overlaps the AllGather) ----
        xe_sb = xpool.tile([128, KD, RPC], BF16)
        enc_phase(xe_sb, 0, W2_ks, b2c_sb, h_ks, MPC, RPC)

        # x2 broadcast tile: x2b[p, rc, r] = |xe_r|^2 for every partition
        sqxe_sb = xpool.tile([128, KD, RPC], BF16, tag="h_all")
        nc.gpsimd.tensor_tensor(sqxe_sb[:, :, :], xe_sb[:, :, :],
                                xe_sb[:, :, :], op=ALU.mult)
        x2row_sb = xpool.tile([1, RPC], F32R)
        x2b_sb = xpool.tile([128, NRC, 512], F32)
        for rc in range(NRC):
            xps = ps.tile([1, 512], F32, tag="tr", bufs=2)
            for k in range(KD):
                nc.tensor.matmul(xps[:], ones_col[:],
                                 sqxe_sb[:, k, 512 * rc:512 * (rc + 1)],
                                 start=(k == 0), stop=(k == KD - 1))
            nc.vector.tensor_copy(x2row_sb[:, 512 * rc:512 * (rc + 1)],
                                  xps[:])
            xbs = ps.tile([128, 512], F32, tag="mm", bufs=4)
            nc.tensor.matmul(xbs[:], ones512[:, :128],
                             x2row_sb[:, 512 * rc:512 * (rc + 1)],
                             start=True, stop=True)
            nc.vector.tensor_copy(x2b_sb[:, rc, :], xbs[:])

        # ---- load gathered anchors: seAg[d_part, k, m] for all 4096 m ----
        # s2 columns first (they gate the first distance tile):
        # s2c[p, t] = s2[128 t + p]
        s2c_sb = gpool.tile([128, NMT], F32)
        for g in range(NCORES):
            rows = ag_out[AGR * g + D:AGR * g + D + 2, :].bitcast(F32)
            nc.scalar.dma_start(
                s2c_sb[:, (MPC // 128) * g:(MPC // 128) * (g + 1)],
                rows.rearrange("a (f p) -> p (a f)", f=2, p=128))
        seAg_gs = []
        for g in range(NCORES):
            sg = gpool.tile([128, KD, MPC], BF16, name=f"seAg{g}")
            for k in range(KD):
                nc.sync.dma_start(
                    sg[:, k, :],
                    ag_out[AGR * g + 128 * k:AGR * g + 128 * (k + 1), :])
            seAg_gs.append(sg)

        # ---- main fused loop over anchor tiles; both row-chunks share
        # each weight load. The zT matmul for tile t is emitted after the
        # d2 group of tile t+2 so the in-order PE stream never waits on
        # the sqrt pass. ----
        zt_ps = [psz.tile([C, 512], F32, name=f"ztps{rc}") for rc in range(NRC)]
        dist_tiles = {}
        for t in range(NMT):
            g, tl = t // (MPC // 128), t % (MPC // 128)
            for rc in range(NRC):
                d2ps = ps.tile([128, 512], F32, tag="mm", bufs=4)
                for k in range(KD):
                    nc.tensor.matmul(d2ps[:],
                                     seAg_gs[g][:, k, 128 * tl:128 * (tl + 1)],
                                     xe_sb[:, k, 512 * rc:512 * (rc + 1)],
                                     start=(k == 0), stop=(k == KD - 1))
                # d2 += s2[m] (per-partition) + x2[r] (broadcast row), fp32
                nc.vector.scalar_tensor_tensor(
                    d2ps[:], d2ps[:], s2c_sb[:, t:t + 1], x2b_sb[:, rc, :],
                    op0=ALU.add, op1=ALU.add)
                distT = dpool.tile([128, 512], F32R)
                nc.scalar.activation(distT[:], d2ps[:], AF.Sqrt)
                dist_tiles[(t, rc)] = distT
            if t >= 2:
                for rc in range(NRC):
                    nc.tensor.matmul(zt_ps[rc][:], Wp_sb[:, t - 2, :],
                                     dist_tiles.pop((t - 2, rc))[:],
                                     start=(t - 2 == 0), stop=False,
                                     skip_group_check=True)
        for t in (NMT - 2, NMT - 1):
            for rc in range(NRC):
                nc.tensor.matmul(zt_ps[rc][:], Wp_sb[:, t, :],
                                 dist_tiles.pop((t, rc))[:],
                                 start=False, stop=False,
                                 skip_group_check=True)

        # ---- epilogue: bias, transpose, then one batched tanh +
        # log-softmax pass (tanh output is in [-1,1] so no max-subtraction
        # is needed) ----
        zpre_sb = zpool.tile([128, 2 * NRC * 2, C], BF16, bufs=1)
        for rc in range(NRC):
            nc.tensor.matmul(zt_ps[rc][:], bp_sb[:], ones512[:],
                             start=False, stop=True, skip_group_check=True)
            zt_sb = zpool.tile([C, 512], F32, bufs=2, tag="zt")
            nc.vector.tensor_copy(zt_sb[:], zt_ps[rc][:])
            for j in range(4):
                ztr = ps.tile([128, C], F32, tag="tr", bufs=2)
                nc.tensor.matmul(ztr[:], zt_sb[:, 128 * j:128 * (j + 1)],
                                 ident[:], is_transpose=True)
                nc.vector.tensor_copy(zpre_sb[:, 4 * rc + j, :], ztr[:])
        NT = 2 * NRC * 2  # 8 tiles of 128 rows
        zth_sb = zpool.tile([128, NT, C], BF16, bufs=1)
        nc.scalar.activation(zth_sb[:, :, :], zpre_sb[:, :, :], AF.Tanh)
        e_sb = zpool.tile([128, NT, C], BF16, bufs=1, tag="zpre_sb")
        nc.scalar.activation(e_sb[:, :, :], zth_sb[:, :, :], AF.Exp)
        ssum = zpool.tile([128, NT], F32, bufs=1)
        nc.vector.tensor_reduce(ssum[:], e_sb[:, :, :],
                                axis=mybir.AxisListType.X, op=ALU.add)
        lns = zpool.tile([128, NT], F32, bufs=1)
        nc.scalar.activation(lns[:], ssum[:], AF.Ln)
        for jj in range(NT):
            o_sb = zpool.tile([128, C], F32, bufs=2, tag="osb")
            nc.vector.tensor_scalar(o_sb[:], zth_sb[:, jj, :],
                                    lns[:, jj:jj + 1], None,
                                    op0=ALU.subtract)
            nc.sync.dma_start(out[128 * jj:128 * (jj + 1), :], o_sb[:])


_NC_CACHE = None


def _get_nc():
    global _NC_CACHE
    if _NC_CACHE is None:
        _NC_CACHE = build_kernel()
    return _NC_CACHE


def make_in_maps(x, samples, W1, b1, W2, b2, Wp, bp):
    bf = ml_dtypes.bfloat16
    x = np.asarray(x, dtype=np.float32)
    samples = np.asarray(samples, dtype=np.float32)
    W1b = np.ascontiguousarray(np.asarray(W1, dtype=np.float32).astype(bf))
    W2b = np.ascontiguousarray(np.asarray(W2, dtype=np.float32).astype(bf))
    Wpc = np.ascontiguousarray(np.asarray(Wp, dtype=np.float32))
    b1c = np.ascontiguousarray(np.asarray(b1, dtype=np.float32).reshape(D, 1))
    b2c = np.ascontiguousarray(np.asarray(b2, dtype=np.float32).reshape(D, 1))
    bpc = np.ascontiguousarray(np.asarray(bp, dtype=np.float32).reshape(1, C))
    in_maps = []
    for g in range(NCORES):
        sT_g = samples[MPC * g:MPC * (g + 1), :].T
        xT_g = x[RPC * g:RPC * (g + 1), :].T
        eT_g = np.concatenate([sT_g, xT_g], axis=1).astype(bf)
        in_maps.append({
            "eT": np.ascontiguousarray(eT_g),
            "W1": W1b, "W2": W2b, "b1": b1c, "b2": b2c,
            "Wp": Wpc, "bp": bpc,
        })
    return in_maps


def run(in_maps, trace=False):
    nc = _get_nc()
    res = bass_utils.run_bass_kernel_spmd(nc, in_maps,
                                          core_ids=list(range(NCORES)),
                                          trace=trace)
    outp = np.concatenate([res.results[g]["out"] for g in range(NCORES)],
                          axis=0).astype(np.float32)
    return outp, res


def kernel(x, samples, W1, b1, W2, b2, Wp, bp):
    in_maps = make_in_maps(x, samples, W1, b1, W2, b2, Wp, bp)
    outp, _ = run(in_maps, trace=False)
    return outp
